# revision 16
# baseline (speedup 1.0000x reference)
"""Complex-magnitude MaxPool2d (k=2, s=2) Trainium2 Bass kernel.

Input  x:  [16, 2, 64, 224, 224] f32  (plane 0 = real, plane 1 = imag)
Output:    [16, 2, 64, 112, 112] f32  (value of the window element with the
                                       largest |z|^2 = re^2 + im^2)

Sharding: pure data parallel over batch: 16 / 8 cores = 2 examples per core.
Per core the 2(batch) x 64(channel) = 128 image planes map 1:1 onto the 128
SBUF partitions; DMA streams 18 row-chunks (6+8 rows at each end to shrink
the pipeline fill/drain bubbles, 14-row chunks in the middle), prefetched
two-deep so DVE never waits on DMA or ACT.

DVE work is compressed with custom fused DVE ops (per-NEFF uop table):
  SQADD   nrm  = re^2 + im^2                  (one pass; kills the ACT Square
                                               pass and the separate DVE add)
  SIGNSEL smax = sel(nE>=nO, -nE, nO)         (H-compare and H-max in one op:
                                               sign bit = "left/even wins",
                                               magnitude = winning norm)
  SQGE    cV   = sq(smaxT) >= sq(smaxB)       (V-compare on |smax| via squares)

The H-select mask (u8, nonzero = even/left wins <=> smax < 0) is derived on
the idle ScalarE: Sign(-smax) in {-1,0,+1} then Relu -> {0,1}, exact in u8.
Selection reproduces jnp.argmax's first-index tie-break: left wins H ties
(is_ge), top wins V ties; norms computed as fl(fl(re^2)+fl(im^2)) on IEEE f32
ALUs, bit-identical to the reference.

Engine split per chunk: DVE: SQADD, SIGNSEL, SQGE, predicated H/V selects.
ScalarE: H prefill (odd cols), mask extract, V prefill (bottom rows).
Pipeline skew: predH runs one chunk behind, predV two behind, so every DVE op
only depends on work finished in earlier iterations.  Middle chunks store in
14-output-row pairs; the four small end chunks flush individually (through
the strided stage view, whose layout is [ri, 14, WO]).
"""

import re as _re

import numpy as np

import concourse.bass as bass
import concourse.mybir as mybir
from concourse import bacc, bass_utils, tile
from concourse import dve_ops as _dvo
from concourse.dve_spec import Spec as _Spec, Src0 as _S0, Src1 as _S1
from concourse.dve_spec import sq as _sq, select as _sel

# Per-core shard geometry (hardcoded; kernel.py must be self-contained).
NCORES = 8
B = 2            # batch per core
RI = 2           # real/imag planes
C = 64           # channels
H = W = 224
HO, WO = H // 2, W // 2
P = 128          # SBUF partitions = B * C
RD = 14          # max image rows per chunk (tile size)
# small chunks at both ends shrink the pipeline-fill and drain bubbles
CH = [(0, 6), (6, 8)] + [(14 + 14 * i, 14) for i in range(14)] + [(210, 8), (218, 6)]
NCHUNK = len(CH)  # 18
N = RD * W       # free elements per plane per full chunk (3136)

F32 = mybir.dt.float32
U8 = mybir.dt.uint8
ACTF = mybir.ActivationFunctionType


def _reg(name, spec):
    """Register a custom DVE op, self-pinning its uops sha."""
    for o in _dvo.OPS:
        if o.name == name:
            return o
    op = _dvo.DveOp(name=name, spec=spec, subdim=False, uops_sha={})
    _dvo.OPS.append(op)
    _dvo.CUSTOM_DVE_SPECS[name] = spec
    _dvo._SUB_OPCODE_FOR_NAME[name] = _dvo._CUSTOM_DVE_ROW_BASE + len(_dvo.OPS) - 1
    assert max(_dvo._SUB_OPCODE_FOR_NAME.values()) < 0x20
    for ver in ("v3", "v4"):
        try:
            op.compile(ver)
        except ValueError as e:
            m = _re.search(r'uops_sha\["' + ver + r'"\]="([0-9a-f]+)"', str(e))
            if not m:
                raise
            op.uops_sha[ver] = m.group(1)
            op.compile(ver)
    return op


SQADD = _reg(
    "ANT_MP_SQADD",
    _Spec(
        body=_sq(_S0) + _sq(_S1),
        reference=lambda in0, in1, s0, s1, imm2: (
            in0.astype(np.float32) * in0 + in1.astype(np.float32) * in1
        ),
    ),
)
SIGNSEL = _reg(
    "ANT_MP_SIGNSEL",
    _Spec(
        body=_sel(_S0 >= _S1, -_S0, _S1),
        reference=lambda in0, in1, s0, s1, imm2: np.where(
            in0 >= in1, -in0, in1
        ).astype(np.float32),
    ),
)
SQGE = _reg(
    "ANT_MP_SQGE",
    _Spec(
        body=_sq(_S0) >= _sq(_S1),
        reference=lambda in0, in1, s0, s1, imm2: (
            in0.astype(np.float32) * in0 >= in1.astype(np.float32) * in1
        ).astype(np.float32),
    ),
)

_NC_CACHE = []


def _build_nc() -> bass.Bass:
    nc = bacc.Bacc("TRN2", target_bir_lowering=False, debug=False)
    # host pre-transposed: partition-major [b*c, ri, H, W] so every DMA is a
    # single-dim 128-partition transfer (hits all 16 SBUF AXI ports)
    x = nc.dram_tensor("x", [P, RI, H, W], F32, kind="ExternalInput").ap()
    out = nc.dram_tensor("out", [P, RI, HO, WO], F32, kind="ExternalOutput").ap()

    with tile.TileContext(nc) as tc:
        with tc.tile_pool(name="p4", bufs=4) as p4, \
             tc.tile_pool(name="p3", bufs=3) as p3, \
             tc.tile_pool(name="p2", bufs=2) as p2, \
             tc.tile_pool(name="p1", bufs=1) as p1:

            xri_t, riH_t, cV_t, mH_t, stage_t = {}, {}, {}, {}, {}

            def rows(k):
                return CH[k][1]

            def dma_in(k):
                r0, rr = CH[k]
                xri = p4.tile([P, RI * N], F32, tag="xri")
                xri_t[k] = xri
                nc.sync.dma_start(
                    out=xri.rearrange("p (ri f) -> p ri f", ri=RI)[:, :, : rr * W],
                    in_=x[:, :, r0 : r0 + rr, :].rearrange("p ri r w -> p ri (r w)"),
                )

            def xri6(k):
                rr = rows(k)
                return xri_t[k].rearrange("p (ri f) -> p ri f", ri=RI)[
                    :, :, : rr * W
                ].rearrange("p ri (r w t) -> p ri r w t", r=rr, w=WO, t=2)

            def riH4(k):
                rr = rows(k)
                return riH_t[k][:, : RI * rr * WO].rearrange(
                    "p (ri r w) -> p ri r w", ri=RI, r=rr, w=WO
                )

            def riH5(k):
                rr = rows(k)
                return riH_t[k][:, : RI * rr * WO].rearrange(
                    "p (ri rp rt w) -> p ri rp rt w", ri=RI, rp=rr // 2, rt=2, w=WO
                )

            # middle full-size chunks are stored in pairs; the four small
            # end chunks flush individually (short fill, short drain)
            def paired(j):
                return 2 <= j <= 15

            def stage_dst4(j):
                st, off, n = stage_t[j]
                return st.rearrange(
                    "p (ri r w) -> p ri r w", ri=RI, r=RD, w=WO
                )[:, :, off : off + n, :]

            for k in range(NCHUNK):
                if k == 0:
                    dma_in(0)
                    dma_in(1)
                if k + 2 < NCHUNK:
                    dma_in(k + 2)
                rr = rows(k)

                # ACT: prefill H-losers (odd cols) straight from the input
                riH = p3.tile([P, RI * RD * WO], F32, tag="riH")
                riH_t[k] = riH
                nc.scalar.copy(out=riH4(k), in_=xri6(k)[:, :, :, :, 1])

                # DVE: nrm = re^2 + im^2 (fused, one pass over the chunk)
                nrm = p1.tile([P, N], F32, tag="nrm")
                xrr = xri_t[k].rearrange("p (ri f) -> p ri f", ri=RI)
                nc.vector._custom_dve(
                    SQADD,
                    out=nrm[:, : rr * W],
                    in0=xrr[:, 0, : rr * W],
                    in1=xrr[:, 1, : rr * W],
                )

                # DVE: smax = sel(nE>=nO, -nE, nO): sign=mask, |.|=H-max
                smax = p2.tile([P, RD * WO], F32, tag="smax")
                nrm_t = nrm[:, : rr * W].rearrange("p (x t) -> p x t", t=2)
                nc.vector._custom_dve(
                    SIGNSEL,
                    out=smax[:, : rr * WO],
                    in0=nrm_t[:, :, 0],
                    in1=nrm_t[:, :, 1],
                )

                # ACT: mH u8 = relu(sign(-smax)) in {0,1}; 1 <=> even/left won
                mHs = p1.tile([P, RD * WO], F32, tag="mHs")
                nc.scalar.activation(
                    out=mHs[:, : rr * WO], in_=smax[:, : rr * WO],
                    func=ACTF.Sign, scale=-1.0,
                )
                mH = p2.tile([P, RD * WO], U8, tag="mH")
                mH_t[k] = mH
                nc.scalar.activation(
                    out=mH[:, : rr * WO], in_=mHs[:, : rr * WO], func=ACTF.Relu
                )

                # DVE: cV = sq(smaxT) >= sq(smaxB)  (|smaxT| >= |smaxB|)
                cV = p3.tile([P, (RD // 2) * WO], U8, tag="cV")
                cV_t[k] = cV
                sm5 = smax[:, : rr * WO].rearrange(
                    "p (rp rt w) -> p rp rt w", rp=rr // 2, rt=2, w=WO
                )
                cv3 = cV[:, : (rr // 2) * WO].rearrange(
                    "p (r w) -> p r w", r=rr // 2, w=WO
                )
                nc.vector._custom_dve(
                    SQGE, out=cv3, in0=sm5[:, :, 0, :], in1=sm5[:, :, 1, :]
                )

                def predh(j):
                    rj = rows(j)
                    # flat 3D APs (out/mask/data) — cheaper AP walk than 4D
                    o3 = riH_t[j][:, : RI * rj * WO].rearrange(
                        "p (ri f) -> p ri f", ri=RI
                    )
                    mb = mH_t[j][:, : rj * WO].unsqueeze(1).broadcast_to(
                        [P, RI, rj * WO]
                    )
                    d3 = xri_t[j].rearrange("p (ri f) -> p ri f", ri=RI)[
                        :, :, : rj * W
                    ].rearrange("p ri (f t) -> p ri f t", t=2)[:, :, :, 0]
                    nc.vector.copy_predicated(out=o3, mask=mb, data=d3)
                    del xri_t[j], mH_t[j]

                def vpre(j):
                    rj = rows(j)
                    if paired(j):
                        if (j - 2) % 2 == 0:
                            st = p2.tile([P, RI * RD * WO], F32, tag="stage",
                                         name="stage")
                        else:
                            st = stage_t[j - 1][0]
                        stage_t[j] = (st, ((j - 2) % 2) * (RD // 2), rj // 2)
                    else:
                        st = p2.tile([P, RI * RD * WO], F32, tag="stage",
                                     name="stage")
                        stage_t[j] = (st, 0, rj // 2)
                    nc.scalar.copy(out=stage_dst4(j), in_=riH5(j)[:, :, :, 1, :])

                def predv(j):
                    rj = rows(j)
                    cb = cV_t[j][:, : (rj // 2) * WO].rearrange(
                        "p (r w) -> p r w", r=rj // 2, w=WO
                    ).unsqueeze(1).broadcast_to([P, RI, rj // 2, WO])
                    nc.vector.copy_predicated(
                        out=stage_dst4(j), mask=cb, data=riH5(j)[:, :, :, 0, :]
                    )
                    del riH_t[j], cV_t[j]
                    flush = (not paired(j)) or ((j - 2) % 2 == 1)
                    if flush:
                        st = stage_t[j][0]
                        if paired(j):
                            o0 = CH[j - 1][0] // 2
                            src = st.rearrange("p (ri f) -> p ri f", ri=RI)
                            n = RD
                        else:
                            o0 = CH[j][0] // 2
                            n = rj // 2
                            # tile layout is [ri, RD, WO]; a short slab must
                            # be read through the strided view
                            src = stage_dst4(j).rearrange("p ri r w -> p ri (r w)")
                        nc.sync.dma_start(
                            out=out[:, :, o0 : o0 + n, :].rearrange(
                                "p ri r w -> p ri (r w)"
                            ),
                            in_=src,
                        )

                if k >= 1:
                    predh(k - 1)
                    vpre(k - 1)
                if k >= 2:
                    predv(k - 2)

            predh(NCHUNK - 1)
            predv(NCHUNK - 2)
            vpre(NCHUNK - 1)
            predv(NCHUNK - 1)
    nc.compile()
    return nc


def get_nc() -> bass.Bass:
    if not _NC_CACHE:
        _NC_CACHE.append(_build_nc())
    return _NC_CACHE[0]


def kernel(x: np.ndarray, **run_kwargs) -> np.ndarray:
    nc = get_nc()
    xs = np.asarray(x, dtype=np.float32)
    assert xs.shape == (NCORES * B, RI, C, H, W), xs.shape
    # [16,2,64,H,W] -> per core [b,c,ri,H,W] flattened to [128,ri,H,W]
    xt = np.ascontiguousarray(xs.transpose(0, 2, 1, 3, 4))
    in_maps = [
        {"x": xt[B * i : B * (i + 1)].reshape(P, RI, H, W)} for i in range(NCORES)
    ]
    res = bass_utils.run_bass_kernel_spmd(
        nc, in_maps, core_ids=list(range(NCORES)), **run_kwargs
    )
    # per-core [128,ri,HO,WO] -> [b,c,ri,HO,WO] -> [b,ri,c,HO,WO]
    out = np.concatenate(
        [
            res.results[i]["out"].reshape(B, C, RI, HO, WO).transpose(0, 2, 1, 3, 4)
            for i in range(NCORES)
        ],
        axis=0,
    )
    if run_kwargs:
        kernel.last_results = res
    return np.ascontiguousarray(out)


# revision 19
# speedup vs baseline: 1.0157x; 1.0157x over previous
"""Complex-magnitude MaxPool2d (k=2, s=2) Trainium2 Bass kernel.

Input  x:  [16, 2, 64, 224, 224] f32  (plane 0 = real, plane 1 = imag)
Output:    [16, 2, 64, 112, 112] f32  (value of the window element with the
                                       largest |z|^2 = re^2 + im^2)

Sharding: pure data parallel over batch: 16 / 8 cores = 2 examples per core.
Per core the 2(batch) x 64(channel) = 128 image planes map 1:1 onto the 128
SBUF partitions; DMA streams 18 row-chunks (6+8 rows at each end to shrink
the pipeline fill/drain bubbles, 14-row chunks in the middle), prefetched
two-deep so DVE never waits on DMA or ACT.

DVE work is compressed with custom fused DVE ops (per-NEFF uop table):
  SQADD   nrm  = re^2 + im^2                  (one pass; kills the ACT Square
                                               pass and the separate DVE add)
  SIGNSEL smax = sel(nE>=nO, -nE, nO)         (H-compare and H-max in one op:
                                               sign bit = "left/even wins",
                                               magnitude = winning norm)
  SQGE    cV   = sq(smaxT) >= sq(smaxB)       (V-compare on |smax| via squares)

The H-select mask (u8, nonzero = even/left wins <=> smax < 0) is derived on
the idle ScalarE: Sign(-smax) in {-1,0,+1} then Relu -> {0,1}, exact in u8.
Selection reproduces jnp.argmax's first-index tie-break: left wins H ties
(is_ge), top wins V ties; norms computed as fl(fl(re^2)+fl(im^2)) on IEEE f32
ALUs, bit-identical to the reference.

Engine split per chunk: DVE: SQADD, SIGNSEL, SQGE, predicated H/V selects.
ScalarE: H prefill (odd cols), mask extract, V prefill (bottom rows).
Pipeline skew: predH runs one chunk behind, predV two behind, so every DVE op
only depends on work finished in earlier iterations.  Middle chunks store in
14-output-row pairs; the four small end chunks flush individually (through
the strided stage view, whose layout is [ri, 14, WO]).
"""

import re as _re

import numpy as np

import concourse.bass as bass
import concourse.mybir as mybir
from concourse import bacc, bass_utils, tile
from concourse import dve_ops as _dvo
from concourse.dve_spec import Spec as _Spec, Src0 as _S0, Src1 as _S1
from concourse.dve_spec import sq as _sq, select as _sel

# Per-core shard geometry (hardcoded; kernel.py must be self-contained).
NCORES = 8
B = 2            # batch per core
RI = 2           # real/imag planes
C = 64           # channels
H = W = 224
HO, WO = H // 2, W // 2
P = 128          # SBUF partitions = B * C
RD = 14          # max image rows per chunk (tile size)
# small chunks at both ends shrink the pipeline-fill and drain bubbles
CH = [(0, 6), (6, 8)] + [(14 + 14 * i, 14) for i in range(14)] + [(210, 8), (218, 6)]
NCHUNK = len(CH)  # 18
N = RD * W       # free elements per plane per full chunk (3136)

F32 = mybir.dt.float32
U8 = mybir.dt.uint8
ACTF = mybir.ActivationFunctionType


def _reg(name, spec):
    """Register a custom DVE op, self-pinning its uops sha."""
    for o in _dvo.OPS:
        if o.name == name:
            return o
    op = _dvo.DveOp(name=name, spec=spec, subdim=False, uops_sha={})
    _dvo.OPS.append(op)
    _dvo.CUSTOM_DVE_SPECS[name] = spec
    _dvo._SUB_OPCODE_FOR_NAME[name] = _dvo._CUSTOM_DVE_ROW_BASE + len(_dvo.OPS) - 1
    assert max(_dvo._SUB_OPCODE_FOR_NAME.values()) < 0x20
    for ver in ("v3", "v4"):
        try:
            op.compile(ver)
        except ValueError as e:
            m = _re.search(r'uops_sha\["' + ver + r'"\]="([0-9a-f]+)"', str(e))
            if not m:
                raise
            op.uops_sha[ver] = m.group(1)
            op.compile(ver)
    return op


SQADD = _reg(
    "ANT_MP_SQADD",
    _Spec(
        body=_sq(_S0) + _sq(_S1),
        reference=lambda in0, in1, s0, s1, imm2: (
            in0.astype(np.float32) * in0 + in1.astype(np.float32) * in1
        ),
    ),
)
SIGNSEL = _reg(
    "ANT_MP_SIGNSEL",
    _Spec(
        body=_sel(_S0 >= _S1, -_S0, _S1),
        reference=lambda in0, in1, s0, s1, imm2: np.where(
            in0 >= in1, -in0, in1
        ).astype(np.float32),
    ),
)
SQGE = _reg(
    "ANT_MP_SQGE",
    _Spec(
        body=_sq(_S0) >= _sq(_S1),
        reference=lambda in0, in1, s0, s1, imm2: (
            in0.astype(np.float32) * in0 >= in1.astype(np.float32) * in1
        ).astype(np.float32),
    ),
)

_NC_CACHE = []


def _build_nc() -> bass.Bass:
    nc = bacc.Bacc("TRN2", target_bir_lowering=False, debug=False)
    # host pre-transposed: partition-major [b*c, ri, H, W] so every DMA is a
    # single-dim 128-partition transfer (hits all 16 SBUF AXI ports)
    x = nc.dram_tensor("x", [P, RI, H, W], F32, kind="ExternalInput").ap()
    out = nc.dram_tensor("out", [P, RI, HO, WO], F32, kind="ExternalOutput").ap()

    with tile.TileContext(nc) as tc:
        with tc.tile_pool(name="p4", bufs=4) as p4, \
             tc.tile_pool(name="p3", bufs=3) as p3, \
             tc.tile_pool(name="p2", bufs=2) as p2, \
             tc.tile_pool(name="p1", bufs=1) as p1:

            xri_t, riH_t, cV_t, mH_t, stage_t = {}, {}, {}, {}, {}

            def rows(k):
                return CH[k][1]

            def dma_in(k):
                r0, rr = CH[k]
                xri = p4.tile([P, RI * N], F32, tag="xri")
                xri_t[k] = xri
                nc.sync.dma_start(
                    out=xri.rearrange("p (ri f) -> p ri f", ri=RI)[:, :, : rr * W],
                    in_=x[:, :, r0 : r0 + rr, :].rearrange("p ri r w -> p ri (r w)"),
                )

            def xri6(k):
                rr = rows(k)
                return xri_t[k].rearrange("p (ri f) -> p ri f", ri=RI)[
                    :, :, : rr * W
                ].rearrange("p ri (r w t) -> p ri r w t", r=rr, w=WO, t=2)

            def riH4(k):
                rr = rows(k)
                return riH_t[k][:, : RI * rr * WO].rearrange(
                    "p (ri r w) -> p ri r w", ri=RI, r=rr, w=WO
                )

            def riH5(k):
                rr = rows(k)
                return riH_t[k][:, : RI * rr * WO].rearrange(
                    "p (ri rp rt w) -> p ri rp rt w", ri=RI, rp=rr // 2, rt=2, w=WO
                )

            # middle full-size chunks are stored in pairs; the four small
            # end chunks flush individually (short fill, short drain)
            def paired(j):
                return 2 <= j <= 15

            def stage_dst4(j):
                st, off, n = stage_t[j]
                return st.rearrange(
                    "p (ri r w) -> p ri r w", ri=RI, r=RD, w=WO
                )[:, :, off : off + n, :]

            for k in range(NCHUNK):
                if k == 0:
                    dma_in(0)
                    dma_in(1)
                if k + 2 < NCHUNK:
                    dma_in(k + 2)
                rr = rows(k)

                # ACT: prefill H-losers (odd cols) straight from the input
                riH = p3.tile([P, RI * RD * WO], F32, tag="riH")
                riH_t[k] = riH
                nc.scalar.copy(out=riH4(k), in_=xri6(k)[:, :, :, :, 1])

                # DVE: nrm = re^2 + im^2 (fused, one pass over the chunk)
                nrm = p1.tile([P, N], F32, tag="nrm")
                xrr = xri_t[k].rearrange("p (ri f) -> p ri f", ri=RI)
                nc.vector._custom_dve(
                    SQADD,
                    out=nrm[:, : rr * W],
                    in0=xrr[:, 0, : rr * W],
                    in1=xrr[:, 1, : rr * W],
                )

                # DVE: smax = sel(nE>=nO, -nE, nO): sign=mask, |.|=H-max
                smax = p2.tile([P, RD * WO], F32, tag="smax")
                nrm_t = nrm[:, : rr * W].rearrange("p (x t) -> p x t", t=2)
                nc.vector._custom_dve(
                    SIGNSEL,
                    out=smax[:, : rr * WO],
                    in0=nrm_t[:, :, 0],
                    in1=nrm_t[:, :, 1],
                )

                # ACT: mH u8 = relu(sign(-smax)) in {0,1}; 1 <=> even/left won
                mHs = p1.tile([P, RD * WO], F32, tag="mHs")
                nc.scalar.activation(
                    out=mHs[:, : rr * WO], in_=smax[:, : rr * WO],
                    func=ACTF.Sign, scale=-1.0,
                )
                mH = p2.tile([P, RD * WO], U8, tag="mH")
                mH_t[k] = mH
                nc.scalar.activation(
                    out=mH[:, : rr * WO], in_=mHs[:, : rr * WO], func=ACTF.Relu
                )

                # DVE: cV = sq(smaxT) >= sq(smaxB)  (|smaxT| >= |smaxB|)
                cV = p3.tile([P, (RD // 2) * WO], U8, tag="cV")
                cV_t[k] = cV
                sm5 = smax[:, : rr * WO].rearrange(
                    "p (rp rt w) -> p rp rt w", rp=rr // 2, rt=2, w=WO
                )
                cv3 = cV[:, : (rr // 2) * WO].rearrange(
                    "p (r w) -> p r w", r=rr // 2, w=WO
                )
                nc.vector._custom_dve(
                    SQGE, out=cv3, in0=sm5[:, :, 0, :], in1=sm5[:, :, 1, :]
                )

                def predh(j):
                    rj = rows(j)
                    # flat 3D APs (out/mask/data) — cheaper AP walk than 4D
                    o3 = riH_t[j][:, : RI * rj * WO].rearrange(
                        "p (ri f) -> p ri f", ri=RI
                    )
                    mb = mH_t[j][:, : rj * WO].unsqueeze(1).broadcast_to(
                        [P, RI, rj * WO]
                    )
                    d3 = xri_t[j].rearrange("p (ri f) -> p ri f", ri=RI)[
                        :, :, : rj * W
                    ].rearrange("p ri (f t) -> p ri f t", t=2)[:, :, :, 0]
                    nc.vector.copy_predicated(out=o3, mask=mb, data=d3)
                    del xri_t[j], mH_t[j]

                def vpre(j):
                    rj = rows(j)
                    if paired(j):
                        if (j - 2) % 2 == 0:
                            st = p2.tile([P, RI * RD * WO], F32, tag="stage",
                                         name="stage")
                        else:
                            st = stage_t[j - 1][0]
                        stage_t[j] = (st, ((j - 2) % 2) * (RD // 2), rj // 2)
                    else:
                        st = p2.tile([P, RI * RD * WO], F32, tag="stage",
                                     name="stage")
                        stage_t[j] = (st, 0, rj // 2)
                    nc.scalar.copy(out=stage_dst4(j), in_=riH5(j)[:, :, :, 1, :])

                def predv(j):
                    rj = rows(j)
                    cb = cV_t[j][:, : (rj // 2) * WO].rearrange(
                        "p (r w) -> p r w", r=rj // 2, w=WO
                    ).unsqueeze(1).broadcast_to([P, RI, rj // 2, WO])
                    nc.vector.copy_predicated(
                        out=stage_dst4(j), mask=cb, data=riH5(j)[:, :, :, 0, :]
                    )
                    del riH_t[j], cV_t[j]
                    flush = (not paired(j)) or ((j - 2) % 2 == 1)
                    if flush:
                        st = stage_t[j][0]
                        if paired(j):
                            o0 = CH[j - 1][0] // 2
                            src = st.rearrange("p (ri f) -> p ri f", ri=RI)
                            n = RD
                        else:
                            o0 = CH[j][0] // 2
                            n = rj // 2
                            # tile layout is [ri, RD, WO]; a short slab must
                            # be read through the strided view
                            src = stage_dst4(j).rearrange("p ri r w -> p ri (r w)")
                        nc.sync.dma_start(
                            out=out[:, :, o0 : o0 + n, :].rearrange(
                                "p ri r w -> p ri (r w)"
                            ),
                            in_=src,
                        )

                if k >= 1:
                    predh(k - 1)
                    vpre(k - 1)
                if k >= 2:
                    predv(k - 2)

            predh(NCHUNK - 1)
            predv(NCHUNK - 2)
            vpre(NCHUNK - 1)
            predv(NCHUNK - 1)
    nc.compile()
    return nc


def get_nc() -> bass.Bass:
    if not _NC_CACHE:
        _NC_CACHE.append(_build_nc())
    return _NC_CACHE[0]


def kernel(x: np.ndarray, **run_kwargs) -> np.ndarray:
    nc = get_nc()
    xs = np.asarray(x, dtype=np.float32)
    assert xs.shape == (NCORES * B, RI, C, H, W), xs.shape
    # [16,2,64,H,W] -> per core [b,c,ri,H,W] flattened to [128,ri,H,W]
    xt = np.ascontiguousarray(xs.transpose(0, 2, 1, 3, 4))
    in_maps = [
        {"x": xt[B * i : B * (i + 1)].reshape(P, RI, H, W)} for i in range(NCORES)
    ]
    res = bass_utils.run_bass_kernel_spmd(
        nc, in_maps, core_ids=list(range(NCORES)), **run_kwargs
    )
    # per-core [128,ri,HO,WO] -> [b,c,ri,HO,WO] -> [b,ri,c,HO,WO]
    out = np.concatenate(
        [
            res.results[i]["out"].reshape(B, C, RI, HO, WO).transpose(0, 2, 1, 3, 4)
            for i in range(NCORES)
        ],
        axis=0,
    )
    if run_kwargs:
        kernel.last_results = res
    return np.ascontiguousarray(out)
